# revision 4
# baseline (speedup 1.0000x reference)
"""DetectionLoss Trainium2 kernel.

Strategy (data-parallel over batch, per sharding hint):
- Shard B=32 across 8 cores (4 images each).
- Host-side prep per core: transpose feature shards to channel-last
  [4*H*W, 144] so each target's 144 channel values are contiguous in DRAM,
  and precompute gather indices / one-hot / mask / DFL weight tensors from
  the (tiny) target tensors.
- Device: one indirect-DMA row gather per 128 targets pulls the 144-float
  feature vector for every (image, target, layer) triple; focal cls loss and
  DFL box loss are computed on-chip (exp/ln on Act engine, reductions and
  elementwise on DVE, partition-sum via PE matmul with a ones vector).
- Host: sum the 8 per-core (cls, box) partials -> (total, cls, box).

The full feature maps are shipped to device DRAM but only the ~250KB/core
actually referenced by the loss is ever read by the kernel, so HW time is far
under the streaming-memory roofline.
"""

import sys
from contextlib import ExitStack

import numpy as np

for _p in ("/opt/trn_rl_repo", "/root/.axon_site/_ro/trn_rl_repo"):
    if _p not in sys.path:
        sys.path.append(_p)

N_CLASSES = 80
N_BINS = 16
B, T = 32, 64
M = 8                       # cores
BL = B // M                 # images per core
C = N_CLASSES + 4 * N_BINS  # 144
HWS = [(80, 80), (40, 40), (20, 20)]
ROWS = 3 * BL * T           # 768 gathered rows per core
NBLK = ROWS // 128          # 6
ROWS_PER_LAYER = BL * T     # 256

_PROG = None


def _build_program():
    import concourse.bass as bass
    import concourse.tile as tile
    from concourse import bacc, mybir

    f32 = mybir.dt.float32
    i32 = mybir.dt.int32
    Act = mybir.ActivationFunctionType
    Alu = mybir.AluOpType

    nc = bacc.Bacc("TRN2", debug=False, num_devices=M)

    feats = [
        nc.dram_tensor(f"f{li}", [BL * h * w, C], f32, kind="ExternalInput").ap()
        for li, (h, w) in enumerate(HWS)
    ]
    idx_d = nc.dram_tensor("idx", [128, NBLK], i32, kind="ExternalInput").ap()
    oh_d = nc.dram_tensor("oh", [128, NBLK, N_CLASSES], f32, kind="ExternalInput").ap()
    msk_d = nc.dram_tensor("msk", [128, NBLK], f32, kind="ExternalInput").ap()
    wd_d = nc.dram_tensor("wd", [128, NBLK, 4 * N_BINS], f32, kind="ExternalInput").ap()
    ws_d = nc.dram_tensor("ws", [128, NBLK, 4], f32, kind="ExternalInput").ap()
    out_d = nc.dram_tensor("out", [1, 2], f32, kind="ExternalOutput").ap()

    with tile.TileContext(nc) as tc, ExitStack() as ctx:
        sb = ctx.enter_context(tc.tile_pool(name="sb", bufs=1))
        ps = ctx.enter_context(tc.tile_pool(name="ps", bufs=1, space="PSUM"))

        idx = sb.tile([128, NBLK], i32)
        oh = sb.tile([128, NBLK, N_CLASSES], f32)
        msk = sb.tile([128, NBLK], f32)
        wd = sb.tile([128, NBLK, 4 * N_BINS], f32)
        ws = sb.tile([128, NBLK, 4], f32)
        nc.sync.dma_start(out=idx[:], in_=idx_d)
        nc.sync.dma_start(out=oh[:], in_=oh_d)
        nc.sync.dma_start(out=msk[:], in_=msk_d)
        nc.sync.dma_start(out=wd[:], in_=wd_d)
        nc.sync.dma_start(out=ws[:], in_=ws_d)

        # Gather each target's class logits (80) and dist logits (64) into
        # separate contiguous tiles; element_offset selects the channel slice
        # within the 144-float channel-last row.
        XC = sb.tile([128, NBLK, N_CLASSES], f32)
        DC = sb.tile([128, NBLK, 4 * N_BINS], f32)
        for blk in range(NBLK):
            nc.gpsimd.indirect_dma_start(
                out=XC[:, blk, :],
                out_offset=None,
                in_=feats[blk // 2],
                in_offset=bass.IndirectOffsetOnAxis(ap=idx[:, blk : blk + 1], axis=0),
                element_offset=4 * N_BINS,
            )
            nc.gpsimd.indirect_dma_start(
                out=DC[:, blk, :],
                out_offset=None,
                in_=feats[blk // 2],
                in_offset=bass.IndirectOffsetOnAxis(ap=idx[:, blk : blk + 1], axis=0),
            )

        X = XC[:]  # [128, 6, 80] class logits
        D = DC[:]  # [128, 6, 64] dist logits

        # ---- focal classification loss ----
        E = sb.tile([128, NBLK, N_CLASSES], f32)
        S = sb.tile([128, NBLK], f32)
        L = sb.tile([128, NBLK], f32)
        TT = sb.tile([128, NBLK, N_CLASSES], f32)
        XS = sb.tile([128, NBLK], f32)
        CE = sb.tile([128, NBLK], f32)
        PT = sb.tile([128, NBLK], f32)
        Q2 = sb.tile([128, NBLK], f32)
        F = sb.tile([128, NBLK], f32)
        FM = sb.tile([128, NBLK], f32)
        P2 = sb.tile([128, 2], f32)

        nc.scalar.activation(out=E[:], in_=X, func=Act.Exp)
        nc.vector.tensor_reduce(out=S[:], in_=E[:], axis=mybir.AxisListType.X, op=Alu.add)
        nc.scalar.activation(out=L[:], in_=S[:], func=Act.Ln)
        nc.vector.tensor_tensor(out=TT[:], in0=X, in1=oh[:], op=Alu.mult)
        nc.vector.tensor_reduce(out=XS[:], in_=TT[:], axis=mybir.AxisListType.X, op=Alu.add)
        nc.vector.tensor_tensor(out=CE[:], in0=L[:], in1=XS[:], op=Alu.subtract)
        nc.scalar.activation(out=PT[:], in_=CE[:], func=Act.Exp, scale=-1.0)
        nc.scalar.activation(out=Q2[:], in_=PT[:], func=Act.Square, scale=-1.0, bias=1.0)
        nc.vector.tensor_tensor(out=F[:], in0=Q2[:], in1=CE[:], op=Alu.mult)
        nc.vector.tensor_tensor(out=FM[:], in0=F[:], in1=msk[:], op=Alu.mult)
        nc.vector.tensor_reduce(
            out=P2[:, 0:1], in_=FM[:], axis=mybir.AxisListType.X, op=Alu.add
        )

        # ---- DFL box loss ----
        # dl = -(lps[lo]*wl + lps[hi]*wr), lps = D - log(sum(exp(D))) per 16-bin
        # group  =>  box = sum(ws * LD) - sum(wd * D) with host-baked sparse
        # weights (ws[.,s] = wl+wr on the selected row/side, wd holds wl/wr at
        # the lo/hi bins).
        ED = sb.tile([128, NBLK, 4 * N_BINS], f32)
        SD = sb.tile([128, NBLK, 4], f32)
        LD = sb.tile([128, NBLK, 4], f32)
        T1 = sb.tile([128, NBLK, 4], f32)
        T2 = sb.tile([128, NBLK, 4 * N_BINS], f32)
        Acc1 = sb.tile([128, 1], f32)
        Acc2 = sb.tile([128, 1], f32)

        nc.scalar.activation(out=ED[:], in_=D, func=Act.Exp)
        nc.vector.tensor_reduce(
            out=SD[:],
            in_=ED[:].rearrange("p r (s n) -> p r s n", n=N_BINS),
            axis=mybir.AxisListType.X,
            op=Alu.add,
        )
        nc.scalar.activation(out=LD[:], in_=SD[:], func=Act.Ln)
        nc.vector.tensor_tensor(out=T1[:], in0=LD[:], in1=ws[:], op=Alu.mult)
        nc.vector.tensor_reduce(
            out=Acc1[:], in_=T1[:], axis=mybir.AxisListType.XY, op=Alu.add
        )
        nc.vector.tensor_tensor(out=T2[:], in0=D, in1=wd[:], op=Alu.mult)
        nc.vector.tensor_reduce(
            out=Acc2[:], in_=T2[:], axis=mybir.AxisListType.XY, op=Alu.add
        )
        nc.vector.tensor_tensor(out=P2[:, 1:2], in0=Acc1[:], in1=Acc2[:], op=Alu.subtract)

        # ---- partition-dim sum via PE: ones[128,1].T @ P2[128,2] -> [1,2] ----
        ONES = sb.tile([128, 1], f32)
        nc.vector.memset(ONES[:], 1.0)
        PS = ps.tile([1, 2], f32)
        nc.tensor.matmul(out=PS[:], lhsT=ONES[:], rhs=P2[:], start=True, stop=True)
        O = sb.tile([1, 2], f32)
        nc.vector.tensor_copy(out=O[:], in_=PS[:])
        nc.sync.dma_start(out=out_d, in_=O[:])

    nc.compile()
    return nc


def _host_prep(feat0, feat1, feat2, tgt_box, tgt_cls, tgt_layer):
    """Build the 8 per-core input maps."""
    f32 = np.float32
    feats = [feat0, feat1, feat2]
    cx, cy = tgt_box[..., 0], tgt_box[..., 1]
    wv, hv = tgt_box[..., 2], tgt_box[..., 3]

    # Per-layer integer grid positions (bit-exact with the f32 reference math).
    fx, fy = {}, {}
    for li, (H, W) in enumerate(HWS):
        fx[li] = np.clip((cx * f32(W)).astype(np.int32), 0, W - 1)  # [B,T]
        fy[li] = np.clip((cy * f32(H)).astype(np.int32), 0, H - 1)

    # Per-layer DFL quantities (the reference's "last matching target" bug).
    tidx = np.arange(T)
    bv = np.arange(B)
    dfl = {}
    for li, (H, W) in enumerate(HWS):
        mask_l = tgt_layer == li
        last = np.max(np.where(mask_l, tidx[None, :], -1), axis=1)  # [B]
        has = last >= 0
        last_c = np.maximum(last, 0)
        lw = np.maximum(wv[bv, last_c], f32(0.0)) * f32(0.5)
        lh = np.maximum(hv[bv, last_c], f32(0.0)) * f32(0.5)
        gt = np.stack([lw * f32(W), lh * f32(H), lw * f32(W), lh * f32(H)], axis=1)
        tq = np.clip(gt, f32(0.0), f32(N_BINS - 1 - 1e-6))
        lo = np.floor(tq)
        wl = (lo + f32(1.0)) - tq
        wr = tq - lo
        lo_i = lo.astype(np.int32)
        hi_i = np.minimum(lo_i + 1, N_BINS - 1)
        dfl[li] = (last_c, has, wl, wr, lo_i, hi_i)

    blv = np.arange(BL)
    in_maps = []
    for m in range(M):
        b0 = m * BL
        sl = slice(b0, b0 + BL)
        im = {}
        for li, (H, W) in enumerate(HWS):
            ft = feats[li][sl].reshape(BL, C, H * W).transpose(0, 2, 1)
            im[f"f{li}"] = np.ascontiguousarray(ft).reshape(BL * H * W, C)

        idx = np.zeros((128, NBLK), np.int32)
        oh = np.zeros((128, NBLK, N_CLASSES), f32)
        msk = np.zeros((128, NBLK), f32)
        wd = np.zeros((128, NBLK, 4 * N_BINS), f32)
        ws = np.zeros((128, NBLK, 4), f32)

        rr_bt = blv[:, None] * T + tidx[None, :]  # [BL, T] row-within-layer
        for li, (H, W) in enumerate(HWS):
            rr = li * ROWS_PER_LAYER + rr_bt
            p, blk = rr % 128, rr // 128
            idx[p, blk] = (
                blv[:, None] * (H * W) + fy[li][sl] * W + fx[li][sl]
            ).astype(np.int32)
            oh[p, blk, tgt_cls[sl]] = f32(1.0)
            msk[p, blk] = (tgt_layer[sl] == li).astype(f32)

            last_c, has, wl, wr, lo_i, hi_i = dfl[li]
            for bl in range(BL):
                b = b0 + bl
                if not has[b]:
                    continue
                r = li * ROWS_PER_LAYER + bl * T + last_c[b]
                p1, blk1 = r % 128, r // 128
                for s in range(4):
                    wd[p1, blk1, s * N_BINS + lo_i[b, s]] = wl[b, s]
                    wd[p1, blk1, s * N_BINS + hi_i[b, s]] = wr[b, s]
                    ws[p1, blk1, s] = wl[b, s] + wr[b, s]

        im["idx"] = idx
        im["oh"] = oh
        im["msk"] = msk
        im["wd"] = wd
        im["ws"] = ws
        in_maps.append(im)
    return in_maps


def kernel(feat0, feat1, feat2, tgt_box, tgt_cls, tgt_layer):
    global _PROG
    from concourse.bass_utils import run_bass_kernel_spmd

    in_maps = _host_prep(feat0, feat1, feat2, tgt_box, tgt_cls, tgt_layer)
    if _PROG is None:
        _PROG = _build_program()
    res = run_bass_kernel_spmd(_PROG, in_maps, list(range(M))).results
    parts = np.stack([res[i]["out"][0] for i in range(M)])  # [M, 2]
    cls_tot = parts[:, 0].sum(dtype=np.float32)
    box_tot = parts[:, 1].sum(dtype=np.float32)
    total = np.float32(cls_tot + box_tot)
    return (total, np.float32(cls_tot), np.float32(box_tot))
